# revision 16
# baseline (speedup 1.0000x reference)
"""Trainium2 Bass kernel for the LocalConnectivity diamond-ring stencil.

out[b, x, y] = sum_{1<=|dx|+|dy|<=5} w[|dx|+|dy|-1] * in[b, (x+dx)%512, (y+dy)%512]

Strategy
--------
Data-parallel over batch: 64 samples -> 8 cores x 8 samples. Per sample the
512x512 grid is processed in 5 row-tiles (~103 output rows each). The whole
60-tap stencil runs on the TensorEngine as 11 PSUM-accumulating matmuls, one
per horizontal shift dy in [-5, 5]:

  psum[p, f] += sum_c  WB_dy[c, p] * X[c, f + dy_idx]

where X is the input tile with 5 halo rows on each side (contraction dim =
nrows+10 partitions) and 5 circular halo columns on each side (horizontal
shifts become free-dim AP offsets), and WB_dy is the banded Toeplitz matrix
holding the vertical taps of kernel column dy: WB_dy[c, p] = K(c-p-5, dy).

float32r keeps the PE at 1 cycle/row while multiplying at FP22 (~2e-4 rel
err). Bulk HBM traffic is issued from GpSimd (software DGE - the only DGE
that fans transfers out across all 16 SDMA engines; the sync/scalar HW DGE
queues serialize on one SDMA engine at ~15-18 GB/s, which is packet-rate
bound at one 2KB row per packet). Transfers stay per-tile so consecutive
DMAs round-robin onto different SDMA queues. Circular column halos are
filled by on-chip ScalarE copies; PSUM eviction alternates Vector/Scalar.
"""

import numpy as np

import concourse.bass as bass
import concourse.bacc as bacc
import concourse.mybir as mybir
from concourse import tile
from concourse.bass_utils import run_bass_kernel_spmd

B, H, W = 64, 512, 512
NCORES = 8
BPC = B // NCORES  # samples per core
MAXD = 5
HALO = MAXD
DYS = 2 * MAXD + 1  # 11 horizontal shifts
TR = 103  # rows per tile (last tile: 100)
ROW_TILES = [(0, 103), (103, 103), (206, 103), (309, 103), (412, 100)]
XW = W + 2 * HALO  # 522


def _build_band_weights(dw: np.ndarray) -> np.ndarray:
    """[128, 11*128] f32: WB[c, j*128 + p] = K(c-p-5, j-5)."""
    wb = np.zeros((128, DYS, 128), dtype=np.float32)
    p = np.arange(128)
    for j in range(DYS):
        dy = j - MAXD
        for dx in range(-MAXD, MAXD + 1):
            d = abs(dx) + abs(dy)
            if 1 <= d <= MAXD:
                c = p + dx + HALO
                valid = (c >= 0) & (c < 128)
                wb[c[valid], j, p[valid]] = dw[d - 1]
    return np.ascontiguousarray(wb.reshape(128, DYS * 128))


_CACHED_NC = None


def _build_program():
    f32 = mybir.dt.float32
    f32r = mybir.dt.float32r

    nc = bacc.Bacc(None, target_bir_lowering=False)
    x = nc.dram_tensor("x", [BPC, H, W], f32r, kind="ExternalInput")
    wb = nc.dram_tensor("wb", [128, DYS * 128], f32r, kind="ExternalInput")
    y = nc.dram_tensor("y", [BPC, H, W], f32, kind="ExternalOutput")

    with tile.TileContext(nc) as tc:
        with (
            tc.tile_pool(name="wpool", bufs=1) as wpool,
            tc.tile_pool(name="xmpool", bufs=4) as xmpool,
            tc.tile_pool(name="xepool", bufs=4) as xepool,
            tc.tile_pool(name="opool", bufs=4) as opool,
            tc.tile_pool(name="pspool", bufs=8, space=bass.MemorySpace.PSUM) as pspool,
        ):
            wtile = wpool.tile([128, DYS * 128], f32r)
            nc.gpsimd.dma_start(wtile[:], wb[:])

            for b in range(BPC):
                # ---- edge tiles t=0 and t=4 (row-wrapped), issued first ----
                xt0 = xepool.tile([128, XW], f32r, tag="xt0")
                #   rows 507..511 then 0..107
                nc.sync.dma_start(
                    xt0[0:HALO, HALO : HALO + W], x[b, H - HALO : H, :]
                )
                nc.gpsimd.dma_start(
                    xt0[HALO : HALO + 108, HALO : HALO + W], x[b, 0:108, :]
                )
                nc.scalar.copy(xt0[0:113, 0:HALO], xt0[0:113, W : W + HALO])
                nc.scalar.copy(xt0[0:113, HALO + W :], xt0[0:113, HALO : 2 * HALO])

                xt4 = xepool.tile([128, XW], f32r, tag="xt4")
                #   rows 407..511 then 0..4
                nc.gpsimd.dma_start(
                    xt4[0:105, HALO : HALO + W], x[b, 4 * TR - HALO : H, :]
                )
                nc.sync.dma_start(xt4[105:110, HALO : HALO + W], x[b, 0:HALO, :])
                nc.scalar.copy(xt4[0:110, 0:HALO], xt4[0:110, W : W + HALO])
                nc.scalar.copy(xt4[0:110, HALO + W :], xt4[0:110, HALO : 2 * HALO])

                # ---- interior tiles t=1..3: per-tile DMAs (each lands on
                # its own SDMA queue; one merged DMA serializes ~0.7MB on a
                # single ~15GB/s queue) ----
                xtm = xmpool.tile([128, 3, XW], f32r)
                for tt in range(3):
                    r0 = TR * (tt + 1)
                    nc.gpsimd.dma_start(
                        xtm[0:113, tt, HALO : HALO + W],
                        x[b, r0 - HALO : r0 + 108, :],
                    )
                    nc.scalar.copy(
                        xtm[0:113, tt, 0:HALO], xtm[0:113, tt, W : W + HALO]
                    )
                    nc.scalar.copy(
                        xtm[0:113, tt, HALO + W :], xtm[0:113, tt, HALO : 2 * HALO]
                    )

                # ---- 11 accumulating matmuls per tile + eviction ----
                otb = opool.tile([128, 5, W], f32)
                for t, (r0, nrows) in enumerate(ROW_TILES):
                    ctr = nrows + 2 * HALO
                    pt = pspool.tile([128, W], f32)
                    for j in range(DYS):
                        if t == 0:
                            rhs = xt0[0:ctr, j : j + W]
                        elif t == 4:
                            rhs = xt4[0:ctr, j : j + W]
                        else:
                            rhs = xtm[0:ctr, t - 1, j : j + W]
                        nc.tensor.matmul(
                            pt[0:nrows, :],
                            wtile[0:ctr, j * 128 : j * 128 + nrows],
                            rhs,
                            start=(j == 0),
                            stop=(j == DYS - 1),
                        )
                    if t % 2 == 0:
                        nc.vector.tensor_copy(otb[0:nrows, t, :], pt[0:nrows, :])
                    else:
                        nc.scalar.copy(otb[0:nrows, t, :], pt[0:nrows, :])

                # ---- per-tile output DMAs (spread across SDMA queues) ----
                for t, (r0, nrows) in enumerate(ROW_TILES):
                    nc.gpsimd.dma_start(
                        y[b, r0 : r0 + nrows, :], otb[0:nrows, t, :]
                    )
    nc.compile()
    return nc


def _get_program():
    global _CACHED_NC
    if _CACHED_NC is None:
        _CACHED_NC = _build_program()
    return _CACHED_NC


def _run(grid_spikes, distance_weights, trace=False):
    grid_spikes = np.ascontiguousarray(np.asarray(grid_spikes, dtype=np.float32))
    distance_weights = np.asarray(distance_weights, dtype=np.float32)
    assert grid_spikes.shape == (B, H, W), grid_spikes.shape
    wb_np = _build_band_weights(distance_weights)

    nc = _get_program()
    in_maps = [
        {
            "x": np.ascontiguousarray(grid_spikes[i * BPC : (i + 1) * BPC]),
            "wb": wb_np,
        }
        for i in range(NCORES)
    ]
    res = run_bass_kernel_spmd(nc, in_maps, list(range(NCORES)), trace=trace)
    out = np.concatenate([res.results[i]["y"] for i in range(NCORES)], axis=0)
    return out.astype(np.float32, copy=False), res


def kernel(grid_spikes, distance_weights):
    out, _ = _run(grid_spikes, distance_weights, trace=False)
    return out


def kernel_traced(grid_spikes, distance_weights):
    out, res = _run(grid_spikes, distance_weights, trace=True)
    return out, res
